# revision 11
# baseline (speedup 1.0000x reference)
import sys

sys.path.insert(0, "/opt/trn_rl_repo")

import numpy as np

from concourse import bass, bacc, tile, mybir
from concourse.bass_utils import run_bass_kernel_spmd

N_CORES = 8
N_PIX = 262144
N_G = 512
PP = 128          # sbuf partitions (pixels per tile)
FT = 256          # pixel tiles per core  -> PP*FT = 32768 pixels/core
NFEAT = 6         # x^2, xy, y^2, x, y, 1
NCHUNK = 8        # featT DMA chunks (overlap load with compute)
NOUT = 4          # output DMA chunks (overlap store with compute)

F32 = mybir.dt.float32


def _build_program():
    nc = bacc.Bacc("TRN2", target_bir_lowering=False, debug=False, num_devices=N_CORES)
    ft_d = nc.dram_tensor("featT", [NFEAT, FT, PP], F32, kind="ExternalInput").ap()
    w_d = nc.dram_tensor("w", [NFEAT, N_G], F32, kind="ExternalInput").ap()
    out_d = nc.dram_tensor("out", [PP, FT], F32, kind="ExternalOutput").ap()

    with tile.TileContext(nc) as tc:
        from contextlib import ExitStack

        with ExitStack() as ctx:
            const = ctx.enter_context(tc.tile_pool(name="const", bufs=1))
            e_pool = ctx.enter_context(tc.tile_pool(name="e", bufs=2, space="PSUM"))
            pq_pool = ctx.enter_context(tc.tile_pool(name="pq", bufs=6, space="PSUM"))

            w_sb = const.tile([NFEAT, N_G], F32)
            acc = const.tile([PP, FT], F32)
            nc.sync.dma_start(w_sb[:], w_d[:])

            CH = FT // NCHUNK
            fts = []
            for ci in range(NCHUNK):
                ft_sb = const.tile([NFEAT, CH, PP], F32)
                nc.sync.dma_start(ft_sb[:], ft_d[:, ci * CH : (ci + 1) * CH, :])
                fts.append(ft_sb)

            OC = FT // NOUT
            for f in range(FT):
                lt = fts[f // CH][:, f % CH, :]
                pq = pq_pool.tile([PP, N_G], F32)
                nc.tensor.matmul(pq[:], lt, w_sb[:], start=True, stop=True)
                e = e_pool.tile([PP, N_G], F32)
                nc.scalar.activation(
                    e[:],
                    pq[:],
                    mybir.ActivationFunctionType.Exp,
                    accum_out=acc[:, f : f + 1],
                )
                if (f + 1) % OC == 0:
                    oi = f // OC
                    nc.sync.dma_start(
                        out_d[:, oi * OC : (oi + 1) * OC],
                        acc[:, oi * OC : (oi + 1) * OC],
                    )
    nc.compile()
    return nc


def _host_weights(alphas, means, rotations, scales):
    a64 = alphas.astype(np.float64)[:, 0]
    m = means.astype(np.float64)
    th = rotations.astype(np.float64)[:, 0]
    s = scales.astype(np.float64)
    c, sn = np.cos(th), np.sin(th)
    s0, s1 = s[:, 0], s[:, 1]
    # M = RS @ RS^T ; cov = inv(M)
    M00 = (s0 * c) ** 2 + (s1 * sn) ** 2
    M01 = (s0 * s0 - s1 * s1) * c * sn
    M11 = (s0 * sn) ** 2 + (s1 * c) ** 2
    det = M00 * M11 - M01 * M01
    A = M11 / det
    B = -M01 / det
    C = M00 / det
    mx, my = m[:, 0], m[:, 1]
    D = -2.0 * (A * mx + B * my)
    E = -2.0 * (B * mx + C * my)
    F0 = A * mx * mx + 2.0 * B * mx * my + C * my * my
    lna = np.log(np.maximum(a64, 1e-300))
    w = np.stack(
        [-A / 2, -B, -C / 2, -D / 2, -E / 2, -F0 / 2 + lna], axis=0
    )  # [6, N]
    return w.astype(np.float32)


def _host_featT(x):
    # featT[c, k, f, p] = feature k of pixel (p*FT + f) on core c
    xc = x.astype(np.float64).reshape(N_CORES, PP, FT, 2)
    xs, ys = xc[..., 0], xc[..., 1]
    feats = np.stack(
        [xs * xs, xs * ys, ys * ys, xs, ys, np.ones_like(xs)], axis=1
    )  # [C, 6, PP, FT]
    return np.ascontiguousarray(feats.transpose(0, 1, 3, 2).astype(np.float32))


_NC_CACHE = {}
LAST_RESULTS = None


def kernel(x, alphas, means, rotations, scales):
    global LAST_RESULTS
    import os

    if "nc" not in _NC_CACHE:
        _NC_CACHE["nc"] = _build_program()
    nc = _NC_CACHE["nc"]

    w = _host_weights(alphas, means, rotations, scales)
    featT = _host_featT(x)
    in_maps = [{"featT": featT[c], "w": w} for c in range(N_CORES)]
    trace = bool(os.environ.get("KERNEL_TRACE"))
    tmpdir = os.environ.get("KERNEL_TRACE_DIR") or None
    res = run_bass_kernel_spmd(
        nc, in_maps, list(range(N_CORES)), trace=trace, tmpdir=tmpdir
    )
    LAST_RESULTS = res
    outs = [res.results[c]["out"].reshape(PP * FT) for c in range(N_CORES)]
    full = np.concatenate(outs, axis=0).reshape(N_PIX, 1).astype(np.float32)
    return full


# revision 14
# speedup vs baseline: 1.2164x; 1.2164x over previous
import sys

sys.path.insert(0, "/opt/trn_rl_repo")

import numpy as np

from concourse import bass, bacc, tile, mybir
from concourse.bass_utils import run_bass_kernel_spmd

N_CORES = 8
N_PIX = 262144
N_G = 512
PP = 128          # sbuf partitions (pixels per tile)
FT = 256          # pixel tiles per core  -> PP*FT = 32768 pixels/core
NFEAT = 6         # x^2, xy, y^2, x, y, 1
NCHUNK = 4        # featT DMA chunks (overlap load with compute)

F32 = mybir.dt.float32


def _build_program():
    nc = bacc.Bacc("TRN2", target_bir_lowering=False, debug=False, num_devices=N_CORES)
    ft_d = nc.dram_tensor("featT", [NFEAT, FT, PP], F32, kind="ExternalInput").ap()
    w_d = nc.dram_tensor("w", [NFEAT, N_G], F32, kind="ExternalInput").ap()
    out_d = nc.dram_tensor("out", [PP, FT], F32, kind="ExternalOutput").ap()

    with tile.TileContext(nc) as tc:
        from contextlib import ExitStack

        with ExitStack() as ctx:
            const = ctx.enter_context(tc.tile_pool(name="const", bufs=1))
            e_pool = ctx.enter_context(tc.tile_pool(name="e", bufs=2, space="PSUM"))
            pq_pool = ctx.enter_context(tc.tile_pool(name="pq", bufs=4, space="PSUM"))

            w_sb = const.tile([NFEAT, N_G], F32)
            acc = const.tile([PP, FT], F32)
            nc.sync.dma_start(w_sb[:], w_d[:])

            CH = FT // NCHUNK
            fts = []
            for ci in range(NCHUNK):
                ft_sb = const.tile([NFEAT, CH, PP], F32)
                nc.sync.dma_start(ft_sb[:], ft_d[:, ci * CH : (ci + 1) * CH, :])
                fts.append(ft_sb)

            for f in range(FT):
                lt = fts[f // CH][:, f % CH, :]
                pq = pq_pool.tile([PP, N_G], F32)
                nc.tensor.matmul(pq[:], lt, w_sb[:], start=True, stop=True)
                e = e_pool.tile([PP, N_G], F32)
                nc.scalar.activation(
                    e[:],
                    pq[:],
                    mybir.ActivationFunctionType.Exp,
                    accum_out=acc[:, f : f + 1],
                )

            nc.sync.dma_start(out_d[:], acc[:])
    nc.compile()
    return nc


def _host_weights(alphas, means, rotations, scales):
    a64 = alphas.astype(np.float64)[:, 0]
    m = means.astype(np.float64)
    th = rotations.astype(np.float64)[:, 0]
    s = scales.astype(np.float64)
    c, sn = np.cos(th), np.sin(th)
    s0, s1 = s[:, 0], s[:, 1]
    # M = RS @ RS^T ; cov = inv(M)
    M00 = (s0 * c) ** 2 + (s1 * sn) ** 2
    M01 = (s0 * s0 - s1 * s1) * c * sn
    M11 = (s0 * sn) ** 2 + (s1 * c) ** 2
    det = M00 * M11 - M01 * M01
    A = M11 / det
    B = -M01 / det
    C = M00 / det
    mx, my = m[:, 0], m[:, 1]
    D = -2.0 * (A * mx + B * my)
    E = -2.0 * (B * mx + C * my)
    F0 = A * mx * mx + 2.0 * B * mx * my + C * my * my
    lna = np.log(np.maximum(a64, 1e-300))
    w = np.stack(
        [-A / 2, -B, -C / 2, -D / 2, -E / 2, -F0 / 2 + lna], axis=0
    )  # [6, N]
    return w.astype(np.float32)


def _host_featT(x):
    # featT[c, k, f, p] = feature k of pixel (p*FT + f) on core c
    xc = x.astype(np.float64).reshape(N_CORES, PP, FT, 2)
    xs, ys = xc[..., 0], xc[..., 1]
    feats = np.stack(
        [xs * xs, xs * ys, ys * ys, xs, ys, np.ones_like(xs)], axis=1
    )  # [C, 6, PP, FT]
    return np.ascontiguousarray(feats.transpose(0, 1, 3, 2).astype(np.float32))


_NC_CACHE = {}
LAST_RESULTS = None


def kernel(x, alphas, means, rotations, scales):
    global LAST_RESULTS
    import os

    if "nc" not in _NC_CACHE:
        _NC_CACHE["nc"] = _build_program()
    nc = _NC_CACHE["nc"]

    w = _host_weights(alphas, means, rotations, scales)
    featT = _host_featT(x)
    in_maps = [{"featT": featT[c], "w": w} for c in range(N_CORES)]
    trace = bool(os.environ.get("KERNEL_TRACE"))
    tmpdir = os.environ.get("KERNEL_TRACE_DIR") or None
    res = run_bass_kernel_spmd(
        nc, in_maps, list(range(N_CORES)), trace=trace, tmpdir=tmpdir
    )
    LAST_RESULTS = res
    outs = [res.results[c]["out"].reshape(PP * FT) for c in range(N_CORES)]
    full = np.concatenate(outs, axis=0).reshape(N_PIX, 1).astype(np.float32)
    return full
